# revision 5
# baseline (speedup 1.0000x reference)
"""Bass/Trainium2 kernel for the attention-GRU decoder step (nn_Decoder).

Model (per reference):
  embedded = emb[input_token]                               # [B, E]
  energy   = tanh([h_last, enc] @ W_attn.T + b_attn)        # [S, B, H]
  scores   = energy @ v_attn ; attn = softmax_S(scores)     # [S, B]
  context  = einsum('sb,sbh->bh', attn, enc)                # [B, H]
  GRU single step (gates r, z, n) -> h_new                  # [B, H]
  logits   = [h_new, context] @ W_fc.T + b_fc               # [B, V]

Sharding over 8 NeuronCores:
  - attention: data-parallel over batch (8 batches/core); encoder slice is
    host-pre-transposed to [H, Bc, S] so the PE contracts over H directly.
  - GRU: tensor-parallel over hidden dim (128 rows/core); requires an
    AllGather of context (batch-sharded -> full) first.
  - fc: tensor-parallel over vocab (4000 cols/core); requires an AllGather
    of h_new (H-sharded -> full) first.
Outputs: logits vocab-sharded (host concat), attn batch-sharded (host
concat), h_new replicated (host takes core 0).
"""

import numpy as np

NCORES = 8
B, S, H, E, V = 64, 512, 1024, 512, 32000
BC, HC, VC = B // NCORES, H // NCORES, V // NCORES
KH = H // 128            # 8 k-tiles over H
KX = (E + H) // 128      # 12 k-tiles over E+H
KF = 2 * H // 128        # 16 k-tiles over 2H
NF = 8                   # fc free-dim chunks
FCW = VC // NF           # 500 columns per fc chunk

# Matmul dtype knobs: "f32" (safe), "f32r" (full-rate fp32, reduced precision),
# "bf16" (data shipped as bf16; halves DMA for enc/W_fc).
ATT_DT = "f32r"
FC_DT = "f32r"

_CACHE = {}


def _split_sync_commands(nc, mybir, max_waits=1, max_updates=1):
    """This walrus build allows one sync wait/update command per instruction.
    Move overflow waits onto preceding NoOps and overflow updates onto
    following NoOps on the same engine queue (same-queue ordering preserves
    semantics)."""
    for fn in nc.m.functions:
        for blk in fn.blocks:
            out = []
            for inst in blk.instructions:
                si = inst.sync_info
                pre, post = [], []
                if si is not None:
                    waits = list(si.on_wait)
                    updates = list(si.on_update)
                    changed = False
                    if len(waits) > max_waits:
                        overflow, keep = waits[:-max_waits], waits[-max_waits:]
                        for i in range(0, len(overflow), max_waits):
                            nop = mybir.InstNoOp(
                                name=nc.get_next_instruction_name(), ins=[], outs=[])
                            nop.engine = inst.engine
                            nop.sync_info = mybir.SyncInfo(
                                on_wait=overflow[i:i + max_waits], on_update=[])
                            nc.register_instruction(nop)
                            pre.append(nop)
                        waits = keep
                        changed = True
                    if len(updates) > max_updates:
                        keep, overflow = updates[:max_updates], updates[max_updates:]
                        for i in range(0, len(overflow), max_updates):
                            nop = mybir.InstNoOp(
                                name=nc.get_next_instruction_name(), ins=[], outs=[])
                            nop.engine = inst.engine
                            nop.sync_info = mybir.SyncInfo(
                                on_wait=[], on_update=overflow[i:i + max_updates])
                            nc.register_instruction(nop)
                            post.append(nop)
                        updates = keep
                        changed = True
                    if changed:
                        inst.sync_info = mybir.SyncInfo(on_wait=waits, on_update=updates)
                out.extend(pre)
                out.append(inst)
                out.extend(post)
            blk.instructions = out


def build_program(att_dt=ATT_DT, fc_dt=FC_DT):
    import concourse.bass as bass
    import concourse.mybir as mybir
    import concourse.tile as tile
    from concourse.masks import make_identity

    f32 = mybir.dt.float32
    bf16 = mybir.dt.bfloat16
    f32r = mybir.dt.float32r

    att_store = bf16 if att_dt == "bf16" else f32
    fc_store = bf16 if fc_dt == "bf16" else f32

    def att_mm(ap):  # matmul-operand view for the attention path
        return ap.bitcast(f32r) if att_dt == "f32r" else ap

    def fc_mm(ap):
        return ap.bitcast(f32r) if fc_dt == "f32r" else ap

    nc = bass.Bass()

    # ---- per-core inputs ----
    enc_in = nc.dram_tensor("enc_t", [H, BC, S], att_store, kind="ExternalInput")
    wa1t_in = nc.dram_tensor("wa1t", [H, H], f32, kind="ExternalInput")
    wa2t_in = nc.dram_tensor("wa2t", [H, H], att_store, kind="ExternalInput")
    hltb_in = nc.dram_tensor("hlt_b", [H, BC], f32, kind="ExternalInput")
    hltf_in = nc.dram_tensor("hlt_full", [H, B], f32, kind="ExternalInput")
    hlthc_in = nc.dram_tensor("hlt_hc", [HC, B], f32, kind="ExternalInput")
    wiht_in = nc.dram_tensor("wiht", [E + H, 3 * HC], f32, kind="ExternalInput")
    whht_in = nc.dram_tensor("whht", [H, 3 * HC], f32, kind="ExternalInput")
    gbias_in = nc.dram_tensor("gbias", [HC, 4], f32, kind="ExternalInput")
    battnt_in = nc.dram_tensor("battnt", [128, KH], f32, kind="ExternalInput")
    vattnt_in = nc.dram_tensor("vattnt", [128, KH], f32, kind="ExternalInput")
    wfct_in = nc.dram_tensor("wfct", [2 * H, VC], fc_store, kind="ExternalInput")
    bfc_in = nc.dram_tensor("bfc", [1, VC], f32, kind="ExternalInput")
    tok_in = nc.dram_tensor("tok", [B, 1], mybir.dt.int32, kind="ExternalInput")
    emb_in = nc.dram_tensor("emb", [V, E], f32, kind="ExternalInput")

    # ---- per-core outputs ----
    logits_out = nc.dram_tensor("logits", [B, VC], f32, kind="ExternalOutput")
    attn_out = nc.dram_tensor("attn", [BC, S], f32, kind="ExternalOutput")
    hnew_out = nc.dram_tensor("h_new", [B, H], f32, kind="ExternalOutput")

    with tile.TileContext(nc) as tc:
        with (
            tc.tile_pool(name="const", bufs=1) as cpool,
            tc.tile_pool(name="wfc", bufs=1) as wfcpool,
            tc.tile_pool(name="dram", bufs=1, space="DRAM") as dram,
        ):
            # ---------- persistent small tiles ----------
            identity = cpool.tile([128, 128], f32, tag="identity")
            make_identity(nc, identity[:])
            ones1 = cpool.tile([1, 128], f32, tag="ones1")
            nc.vector.memset(ones1[:], 1.0)
            ones64 = cpool.tile([1, B], f32, tag="ones64")
            nc.vector.memset(ones64[:], 1.0)

            wa2t_t = []
            for k in range(KH):
                t = cpool.tile([128, H], att_store, tag=f"wa2t{k}")
                nc.sync.dma_start(t[:], wa2t_in[128 * k:128 * (k + 1), :])
                wa2t_t.append(t)
            hltb_t = []
            for k in range(KH):
                t = cpool.tile([128, BC], f32, tag=f"hltb{k}")
                nc.sync.dma_start(t[:], hltb_in[128 * k:128 * (k + 1), :])
                hltb_t.append(t)
            hltf_t = []
            for k in range(KH):
                t = cpool.tile([128, B], f32, tag=f"hltf{k}")
                nc.sync.dma_start(t[:], hltf_in[128 * k:128 * (k + 1), :])
                hltf_t.append(t)
            hlthc = cpool.tile([HC, B], f32, tag="hlthc")
            nc.sync.dma_start(hlthc[:], hlthc_in[:])
            gbias = cpool.tile([HC, 4], f32, tag="gbias")
            nc.sync.dma_start(gbias[:], gbias_in[:])
            battnt = cpool.tile([128, KH], f32, tag="battnt")
            nc.sync.dma_start(battnt[:], battnt_in[:])
            vattnt = cpool.tile([128, KH], f32, tag="vattnt")
            nc.sync.dma_start(vattnt[:], vattnt_in[:])
            bfc_sb = cpool.tile([1, VC], f32, tag="bfc")
            nc.sync.dma_start(bfc_sb[:], bfc_in[:])
            tok_sb = cpool.tile([B, 1], mybir.dt.int32, tag="tok")
            nc.sync.dma_start(tok_sb[:], tok_in[:])

            # hp_sb[m][:, b] = (Wa1 @ h_last_b)[h-tile m] + b_attn[h-tile m]
            hp_sb = [cpool.tile([128, BC], f32, tag=f"hp{m}", name=f"hp{m}") for m in range(KH)]
            # context^T accumulators, [128, BC] columns per local batch
            ctxc = [cpool.tile([128, BC], f32, tag=f"ctxc{k}", name=f"ctxc{k}") for k in range(KH)]
            # full context^T / h_new^T tiles (post-AllGather), fc inputs
            ctxT = [cpool.tile([128, B], f32, tag=f"ctxT{k}", name=f"ctxT{k}") for k in range(KH)]
            hnT = [cpool.tile([128, B], f32, tag=f"hnT{k}", name=f"hnT{k}") for k in range(KH)]

            # DRAM bounce buffers for the collectives
            ag1_in = dram.tile([BC, H], f32, tag="ag1i")
            ag1_out = dram.tile([B, H], f32, tag="ag1o", addr_space="Shared")
            ag2_in = dram.tile([HC, B], f32, tag="ag2i")
            ag2_out = dram.tile([H, B], f32, tag="ag2o", addr_space="Shared")

            # ---------- P0: h_last projection (Wa1T) ----------
            with tc.tile_pool(name="ps0", space="PSUM", bufs=1) as ps0, \
                 tc.tile_pool(name="sb0", bufs=1) as sb0:
                hp_ps = [ps0.tile([128, BC], f32, tag=f"hpp{m}", name=f"hpp{m}") for m in range(KH)]
                for k in range(KH):
                    wa1t_tile = sb0.tile([128, H], f32, tag="wa1t", bufs=2)
                    nc.sync.dma_start(wa1t_tile[:], wa1t_in[128 * k:128 * (k + 1), :])
                    for m in range(KH):
                        nc.tensor.matmul(
                            hp_ps[m][:], wa1t_tile[:, 128 * m:128 * (m + 1)],
                            hltb_t[k][:], start=(k == 0), stop=(k == KH - 1))
                for m in range(KH):
                    nc.vector.tensor_scalar_add(
                        hp_sb[m][:], hp_ps[m][:], battnt[:, m:m + 1])

            # ---------- P1: attention, one local batch at a time ----------
            with tc.tile_pool(name="sbA", bufs=1) as sbA, \
                 tc.tile_pool(name="psA", space="PSUM", bufs=1) as psA:
                enc_tiles = [[None] * KH for _ in range(BC)]
                for b in range(BC):
                    for k in range(KH):
                        et = sbA.tile([128, S], att_store, tag="enc", bufs=18)
                        nc.sync.dma_start(et[:], enc_in[128 * k:128 * (k + 1), b, :])
                        enc_tiles[b][k] = et

                    energy = []
                    for m in range(KH):
                        e_ps = psA.tile([128, S], f32, tag="e", bufs=3)
                        for k in range(KH):
                            nc.tensor.matmul(
                                e_ps[:],
                                att_mm(wa2t_t[k][:, 128 * m:128 * (m + 1)]),
                                att_mm(enc_tiles[b][k][:]),
                                start=(k == 0), stop=(k == KH - 1))
                        e_sb = sbA.tile([128, S], att_store, tag="energy", bufs=10)
                        nc.scalar.activation(
                            e_sb[:], e_ps[:], mybir.ActivationFunctionType.Tanh,
                            bias=hp_sb[m][:, b:b + 1])
                        energy.append(e_sb)

                    s_ps = psA.tile([1, S], f32, tag="s", bufs=2)
                    for m in range(KH):
                        nc.tensor.matmul(
                            s_ps[:], att_mm(vattnt[:, m:m + 1]), att_mm(energy[m][:]),
                            start=(m == 0), stop=(m == KH - 1))

                    # softmax over S on partition 0
                    mx = sbA.tile([1, 1], f32, tag="mx", bufs=2)
                    nc.vector.tensor_reduce(
                        mx[:], s_ps[:], axis=mybir.AxisListType.X,
                        op=mybir.AluOpType.max)
                    nmx = sbA.tile([1, 1], f32, tag="nmx", bufs=2)
                    nc.vector.tensor_scalar_mul(nmx[:], mx[:], -1.0)
                    den = sbA.tile([1, 1], f32, tag="den", bufs=2)
                    ex = sbA.tile([1, S], f32, tag="ex", bufs=2)
                    nc.scalar.activation(
                        ex[:], s_ps[:], mybir.ActivationFunctionType.Exp,
                        bias=nmx[:, 0:1], accum_out=den[:])
                    rden = sbA.tile([1, 1], f32, tag="rden", bufs=2)
                    nc.vector.reciprocal(rden[:], den[:])
                    at_sb = sbA.tile([1, S], f32, tag="at", bufs=2)
                    nc.vector.tensor_scalar_mul(at_sb[:], ex[:], rden[:, 0:1])
                    nc.sync.dma_start(attn_out[b:b + 1, :], at_sb[:])

                    # broadcast attn row to 128 partitions via PE
                    bc_ps = psA.tile([128, S], f32, tag="bc", bufs=2)
                    nc.tensor.matmul(bc_ps[:], ones1[:], at_sb[:],
                                     start=True, stop=True)

                    # context^T[:, b] += sum_s attn[s] * enc_t[h, s]
                    for k in range(KH):
                        scr = sbA.tile([128, S], f32, tag="scr", bufs=3)
                        nc.vector.tensor_tensor(
                            scr[:], enc_tiles[b][k][:], bc_ps[:],
                            mybir.AluOpType.mult)
                        nc.vector.tensor_reduce(
                            ctxc[k][:, b:b + 1], scr[:],
                            axis=mybir.AxisListType.X, op=mybir.AluOpType.add)

            # ---------- P2/P3/P4: gather context, GRU, gather h_new ----------
            with tc.tile_pool(name="sbB", bufs=1) as sbB, \
                 tc.tile_pool(name="psB", space="PSUM", bufs=1) as psB:
                # embedding gather + transpose -> embT tiles [128, B]
                emb_sb = sbB.tile([B, E], f32, tag="embg")
                nc.gpsimd.indirect_dma_start(
                    out=emb_sb[:], out_offset=None, in_=emb_in[:],
                    in_offset=bass.IndirectOffsetOnAxis(ap=tok_sb[:, :1], axis=0))
                embT = []
                for c in range(E // 128):
                    tp = psB.tile([128, B], f32, tag="tp", bufs=2)
                    nc.tensor.transpose(
                        tp[:], emb_sb[:, 128 * c:128 * (c + 1)], identity[:B, :B])
                    te = sbB.tile([128, B], f32, tag=f"embT{c}")
                    nc.scalar.copy(te[:], tp[:])
                    embT.append(te)

                # context^T [128, BC] tiles -> natural [BC, H] -> AllGather
                ctx_nat = sbB.tile([BC, H], f32, tag="ctxnat")
                for k in range(KH):
                    tp = psB.tile([BC, 128], f32, tag="tp", bufs=2)
                    nc.tensor.transpose(tp[:], ctxc[k][:], identity[:])
                    nc.scalar.copy(ctx_nat[:, 128 * k:128 * (k + 1)], tp[:])
                nc.gpsimd.dma_start(ag1_in[:], ctx_nat[:])
                nc.gpsimd.collective_compute(
                    "AllGather", mybir.AluOpType.bypass,
                    ins=[ag1_in.opt()], outs=[ag1_out.opt()],
                    replica_groups=[list(range(NCORES))])
                ctxf_nat = sbB.tile([B, H], f32, tag="ctxfnat")
                nc.sync.dma_start(ctxf_nat[:], ag1_out[:])
                for k in range(KH):
                    tp = psB.tile([128, B], f32, tag="tp", bufs=2)
                    nc.tensor.transpose(
                        tp[:], ctxf_nat[:, 128 * k:128 * (k + 1)], identity[:B, :B])
                    nc.scalar.copy(ctxT[k][:], tp[:])

                # GRU gate matmuls: gi = W_ih @ x, gh = W_hh @ h  (transposed)
                xT = embT + ctxT
                gi_ps = [psB.tile([HC, B], f32, tag=f"gi{g}", name=f"gi{g}") for g in range(3)]
                gh_ps = [psB.tile([HC, B], f32, tag=f"gh{g}", name=f"gh{g}") for g in range(3)]
                for k in range(KX):
                    wt = sbB.tile([128, 3 * HC], f32, tag="wiht", bufs=3)
                    nc.sync.dma_start(wt[:], wiht_in[128 * k:128 * (k + 1), :])
                    for g in range(3):
                        nc.tensor.matmul(
                            gi_ps[g][:], wt[:, HC * g:HC * (g + 1)], xT[k][:],
                            start=(k == 0), stop=(k == KX - 1))
                for k in range(KH):
                    wt = sbB.tile([128, 3 * HC], f32, tag="whht", bufs=3)
                    nc.sync.dma_start(wt[:], whht_in[128 * k:128 * (k + 1), :])
                    for g in range(3):
                        nc.tensor.matmul(
                            gh_ps[g][:], wt[:, HC * g:HC * (g + 1)], hltf_t[k][:],
                            start=(k == 0), stop=(k == KH - 1))

                # gates: r, z, n and h_new^T (all [HC, B])
                AF = mybir.ActivationFunctionType
                ghr = sbB.tile([HC, B], f32, tag="ghr")
                nc.scalar.copy(ghr[:], gh_ps[0][:])
                ghz = sbB.tile([HC, B], f32, tag="ghz")
                nc.scalar.copy(ghz[:], gh_ps[1][:])
                tr = sbB.tile([HC, B], f32, tag="tr")
                nc.vector.tensor_tensor(tr[:], gi_ps[0][:], ghr[:],
                                        mybir.AluOpType.add)
                rT = sbB.tile([HC, B], f32, tag="rT")
                nc.scalar.activation(rT[:], tr[:], AF.Sigmoid, bias=gbias[:, 0:1])
                tz = sbB.tile([HC, B], f32, tag="tz")
                nc.vector.tensor_tensor(tz[:], gi_ps[1][:], ghz[:],
                                        mybir.AluOpType.add)
                zT = sbB.tile([HC, B], f32, tag="zT")
                nc.scalar.activation(zT[:], tz[:], AF.Sigmoid, bias=gbias[:, 1:2])
                hnp = sbB.tile([HC, B], f32, tag="hnp")
                nc.scalar.activation(hnp[:], gh_ps[2][:], AF.Identity,
                                     bias=gbias[:, 3:4])
                tn = sbB.tile([HC, B], f32, tag="tn")
                nc.vector.tensor_tensor(tn[:], rT[:], hnp[:], mybir.AluOpType.mult)
                tn2 = sbB.tile([HC, B], f32, tag="tn2")
                nc.vector.tensor_tensor(tn2[:], gi_ps[2][:], tn[:],
                                        mybir.AluOpType.add)
                nT = sbB.tile([HC, B], f32, tag="nT")
                nc.scalar.activation(nT[:], tn2[:], AF.Tanh, bias=gbias[:, 2:3])
                dH = sbB.tile([HC, B], f32, tag="dH")
                nc.vector.tensor_sub(dH[:], hlthc[:], nT[:])
                zd = sbB.tile([HC, B], f32, tag="zd")
                nc.vector.tensor_tensor(zd[:], zT[:], dH[:], mybir.AluOpType.mult)
                hnewT_c = sbB.tile([HC, B], f32, tag="hnewT")
                nc.vector.tensor_add(hnewT_c[:], nT[:], zd[:])

                # AllGather h_new^T shards -> [H, B]
                nc.gpsimd.dma_start(ag2_in[:], hnewT_c[:])
                nc.gpsimd.collective_compute(
                    "AllGather", mybir.AluOpType.bypass,
                    ins=[ag2_in.opt()], outs=[ag2_out.opt()],
                    replica_groups=[list(range(NCORES))])
                for k in range(KH):
                    nc.sync.dma_start(hnT[k][:], ag2_out[128 * k:128 * (k + 1), :])

                # h_new natural output [B, H]
                hn_nat = sbB.tile([B, H], f32, tag="hnnat")
                for k in range(KH):
                    tp = psB.tile([B, 128], f32, tag="tp", bufs=2)
                    nc.tensor.transpose(tp[:], hnT[k][:], identity[:])
                    nc.scalar.copy(hn_nat[:, 128 * k:128 * (k + 1)], tp[:])
                nc.sync.dma_start(hnew_out[:], hn_nat[:])

            # ---------- P5: fc projection over this core's vocab slice ----------
            with tc.tile_pool(name="sbC", bufs=1) as sbC, \
                 tc.tile_pool(name="psC", space="PSUM", bufs=1) as psC:
                xcat = hnT + ctxT
                fc_ps = [psC.tile([B, FCW], f32, tag=f"fc{n}", name=f"fc{n}") for n in range(NF)]
                for n in range(NF):
                    nc.tensor.matmul(
                        fc_ps[n][:], ones64[:], bfc_sb[:, FCW * n:FCW * (n + 1)],
                        start=True, stop=False)
                for k in range(KF):
                    wt = wfcpool.tile([128, VC], fc_store, tag="wfc", bufs=2)
                    nc.sync.dma_start(wt[:], wfct_in[128 * k:128 * (k + 1), :])
                    for n in range(NF):
                        nc.tensor.matmul(
                            fc_ps[n][:], fc_mm(xcat[k][:]),
                            fc_mm(wt[:, FCW * n:FCW * (n + 1)]),
                            start=False, stop=(k == KF - 1))
                for n in range(NF):
                    lg = sbC.tile([B, FCW], f32, tag="lg", bufs=2)
                    nc.scalar.copy(lg[:], fc_ps[n][:])
                    nc.sync.dma_start(logits_out[:, FCW * n:FCW * (n + 1)], lg[:])

    import concourse.mybir as mybir2
    _split_sync_commands(nc, mybir2)
    return nc


def prepare_inputs(input_token, hidden, encoder_outputs, emb, W_attn, b_attn,
                   v_attn, W_ih, W_hh, b_ih, b_hh, W_fc, b_fc,
                   att_dt=ATT_DT, fc_dt=FC_DT):
    """Shard/transpose the full inputs into 8 per-core input maps."""
    f4 = np.float32
    att_np = np.dtype("bfloat16") if att_dt == "bf16" else f4
    fc_np = np.dtype("bfloat16") if fc_dt == "bf16" else f4

    enc = np.asarray(encoder_outputs, f4)          # [S, B, H]
    wa_t = np.ascontiguousarray(np.asarray(W_attn, f4).T)   # [2H, H]
    wa1t = np.ascontiguousarray(wa_t[:H])
    wa2t = np.ascontiguousarray(wa_t[H:], dtype=att_np)
    hlt = np.ascontiguousarray(np.asarray(hidden, f4)[0].T)  # [H, B]
    W_ih = np.asarray(W_ih, f4)
    W_hh = np.asarray(W_hh, f4)
    b_ih = np.asarray(b_ih, f4)
    b_hh = np.asarray(b_hh, f4)
    wfcT = np.asarray(W_fc, f4).T                   # [2H, V] (view)
    b_fc = np.asarray(b_fc, f4)
    b_attn = np.asarray(b_attn, f4)
    v_attn = np.asarray(v_attn, f4)
    battnt = np.ascontiguousarray(b_attn.reshape(KH, 128).T)
    vattnt = np.ascontiguousarray(v_attn.reshape(KH, 128).T)
    tok = np.asarray(input_token).astype(np.int32).reshape(B, 1)
    emb = np.asarray(emb, f4)

    in_maps = []
    for c in range(NCORES):
        bs = slice(c * BC, (c + 1) * BC)
        hc = slice(c * HC, (c + 1) * HC)
        gidx = np.r_[c * HC:(c + 1) * HC,
                     H + c * HC:H + (c + 1) * HC,
                     2 * H + c * HC:2 * H + (c + 1) * HC]
        gbias = np.stack([
            (b_ih[:H] + b_hh[:H])[hc],
            (b_ih[H:2 * H] + b_hh[H:2 * H])[hc],
            b_ih[2 * H:][hc],
            b_hh[2 * H:][hc],
        ], axis=1)
        in_maps.append({
            "enc_t": np.ascontiguousarray(
                enc[:, bs, :].transpose(2, 1, 0), dtype=att_np),
            "wa1t": wa1t,
            "wa2t": wa2t,
            "hlt_b": np.ascontiguousarray(hlt[:, bs]),
            "hlt_full": hlt,
            "hlt_hc": np.ascontiguousarray(hlt[hc, :]),
            "wiht": np.ascontiguousarray(W_ih[gidx].T),
            "whht": np.ascontiguousarray(W_hh[gidx].T),
            "gbias": np.ascontiguousarray(gbias),
            "battnt": battnt,
            "vattnt": vattnt,
            "wfct": np.ascontiguousarray(wfcT[:, c * VC:(c + 1) * VC], dtype=fc_np),
            "bfc": np.ascontiguousarray(b_fc[c * VC:(c + 1) * VC]).reshape(1, VC),
            "tok": tok,
            "emb": emb,
        })
    return in_maps


def assemble_outputs(results):
    logits = np.concatenate([results[c]["logits"] for c in range(NCORES)], axis=1)
    h_new = results[0]["h_new"].reshape(1, B, H)
    attn = np.concatenate([results[c]["attn"] for c in range(NCORES)], axis=0)
    return logits, h_new, attn


def kernel(input_token, hidden, encoder_outputs, emb, W_attn, b_attn, v_attn,
           W_ih, W_hh, b_ih, b_hh, W_fc, b_fc):
    from concourse.bass_utils import run_bass_kernel_spmd

    key = (ATT_DT, FC_DT)
    if key not in _CACHE:
        _CACHE[key] = build_program(*key)
    nc = _CACHE[key]

    in_maps = prepare_inputs(
        input_token, hidden, encoder_outputs, emb, W_attn, b_attn, v_attn,
        W_ih, W_hh, b_ih, b_hh, W_fc, b_fc)
    res = run_bass_kernel_spmd(nc, in_maps, list(range(NCORES)))
    return assemble_outputs(res.results)
